# revision 23
# baseline (speedup 1.0000x reference)
"""LinearSpline activation kernel for Trainium2 (8 NeuronCores, SPMD).

Math: per channel c the reference is a 51-knot uniform linear spline. With
t = x*scale/grid + 25 it becomes, in the slope-change basis,
    g(t) = A + b*t + sum_{k=1..49} gamma_k * relu(t - k)
(all per-channel constants folded on the host, incl. the 1/scale output
factor). Terms are split across engines:

  DVE   : K3 streamless 3-term partial ops (knots 1..3*K3, max-form
          gamma_a*max(t,k-1)+gamma_b*max(t,k)+gamma_c*max(t,k+1); the
          knot offsets ride imm2 +/- a latched One so no shifted input
          streams are needed)
  ACT   : t-stream, T_A single-knot relu streams relu(a_s*x + (25-k))
          (unweighted; gamma rides the PE diag weights), and the final
          PSUM->SBUF copy which adds the constant A via its bias
  PE    : merges everything into PSUM with fp32r matmuls: diag(b) @ t,
          I @ partial_i, diag(gamma_k) @ relu_k  (accumulating groups
          per 512-col PSUM bank)
  GPSIMD: half of the unit-0 input DMA only

Layout: data-parallel over batch (4 per core), partition p = n2*64 + ch,
tiles [128, 2048], 16 units per core. I/O in fp16 (error << tolerance).
"""

import numpy as np

# ---------------- problem constants (hardcoded; kernel.py is standalone) ---
N_BATCH, N_CH, H, W = 32, 64, 128, 128
NCORES = 8
SIZE = 51
GRID = np.float64(2.0 * 4.0 / (SIZE - 1))  # 0.16
F = 2048                    # free-dim chunk per unit
GROUPS = 2                  # batch pairs per core
FREE = H * W                # 16384 free elems per (group, partition)
UNITS = GROUPS * (FREE // F)    # 16
NB = F // 512               # psum banks per tile

K3 = 12                     # 3-term DVE partials -> knots 1..3*K3
ALT_COPY = True             # alternate PSUM->SBUF copy between ACT and DVE
T_A = 49 - 3 * K3           # ACT relu terms -> knots 40..49
NW = 2 + T_A                # weight mats: diag(b), I, diag(gamma_k)*T_A
RP = 12                     # partial ring depth
RR = 8                      # relu-stream ring depth (fp16 tiles)
RY = 3                      # output-buffer ring depth
RT = 3                      # t-stream ring depth

# PE merge order per unit: seed first, then interleave relu/partial merges
# by earliest availability (ACT produces r_j at ~1.9us apart, DVE partials
# at ~2.2us apart, PE consumes at ~1.1us).
MO = [("s", 0)]
_r, _p = 0, 0
for _n in range(T_A + K3):
    if _r < T_A and (_r <= _p + 1 or _p >= K3):
        MO.append(("r", _r)); _r += 1
    else:
        MO.append(("p", _p)); _p += 1
M = len(MO)                 # merges per unit
MI_R = {j: m for m, (kind, j) in enumerate(MO) if kind == "r"}
MI_P = {i: m for m, (kind, i) in enumerate(MO) if kind == "p"}

_f32, _f64 = np.float32, np.float64
_built = {}


def _host_params(coeff, scal):
    """coeff [3264], scal [64] -> (prm [128, 64] f32, wg [NW, 128, 128] f32).

    prm cols: 0 a_s, 1 t-bias (25), 2 A, 3+3i/4+3i/5+3i partial gammas
    (C0=g_{3i+2}, C1=g_{3i+3}, C3=g_{3i+1}), 42+j relu bias (25-(40+j)).
    wg: [0] diag(b), [1] I, [2+j] diag(gamma_{40+j})."""
    C = coeff.reshape(N_CH, SIZE).astype(_f64)
    d = C[:, 1:] - C[:, :-1]                    # [64, 50]
    s = scal.astype(_f64)
    b = d[:, 0] / s
    gam = (d[:, 1:] - d[:, :-1]) / s[:, None]   # [64, 49]; gamma_k at col k-1
    A = C[:, 25] / s - 25.0 * b
    for k in range(1, 3 * K3 + 1):
        A = A - gam[:, k - 1] * max(25.0, float(k))

    prm = np.zeros((N_CH, 64), _f64)
    prm[:, 0] = s / GRID
    prm[:, 1] = 25.0
    prm[:, 2] = A
    for i in range(K3):
        prm[:, 3 + 3 * i] = gam[:, 3 * i + 1]   # C0 = gamma_{3i+2}
        prm[:, 4 + 3 * i] = gam[:, 3 * i + 2]   # C1 = gamma_{3i+3}
        prm[:, 5 + 3 * i] = gam[:, 3 * i + 0]   # C3 = gamma_{3i+1}
    for j in range(T_A):
        prm[:, 42 + j] = 25.0 - (3 * K3 + 1 + j)
    prm2 = np.tile(prm.astype(_f32), (2, 1))    # [128, 64]

    wg = np.zeros((NW, 128, 128), _f32)
    idx = np.arange(128)
    bb = np.tile(b, 2)
    wg[0, idx, idx] = bb.astype(_f32)
    wg[1, idx, idx] = 1.0
    for j in range(T_A):
        gj = np.tile(gam[:, 3 * K3 + j], 2)
        wg[2 + j, idx, idx] = gj.astype(_f32)
    return prm2, wg


def _register_ops():
    import concourse.dve_ops as dve_ops
    from concourse.dve_spec import (
        Spec, Src0, C0, C1, C2, C3, One, lower, maxx, _spill_c3_to_src1,
        Latch,
    )
    from concourse.dve_uop import DveOpSpec

    def reg(name, spec, rd1):
        for op in dve_ops.OPS:
            if op.name == name:
                return op
        row = max(dve_ops._SUB_OPCODE_FOR_NAME.values()) + 1
        assert row < 0x20
        dve_ops._SUB_OPCODE_FOR_NAME[name] = row
        uops = lower(spec, ver="v3")
        sha = DveOpSpec(name=name, opcode=row, uops=uops, rd1_en=rd1).sha("v3")
        op = dve_ops.DveOp(name, spec, subdim=False, uops_sha={"v3": sha})
        dve_ops.OPS.append(op)
        dve_ops.CUSTOM_DVE_SPECS[name] = spec
        return op

    # streamless 3-term partial:
    #   C0*max(t, k) + C1*max(t, k+1) + C3*max(t, k-1),  k = imm2
    part = reg("LS_P3L", Spec(body=_spill_c3_to_src1(
        C0 * maxx(Src0, C2) + C1 * maxx(Src0, Latch(C2 + One))
        + C3 * maxx(Src0, Latch(C2 - One)))), rd1=True)
    return part


def _build():
    if "nc" in _built:
        return _built["nc"]
    import concourse.bass as bass
    import concourse.mybir as mybir
    from concourse.library_overlay import lower_extended_insts

    P3L = _register_ops()
    F32 = mybir.dt.float32
    F32R = mybir.dt.float32r
    F16 = mybir.dt.float16
    Ident = mybir.ActivationFunctionType.Identity
    Relu = mybir.ActivationFunctionType.Relu

    nc = bass.Bass()
    x_in = nc.declare_dram_parameter("x", [GROUPS, 128, FREE], F16,
                                     isOutput=False)
    prm = nc.declare_dram_parameter("prm", [128, 64], F32, isOutput=False)
    wgt = nc.declare_dram_parameter("wgt", [NW, 128, 128], F32,
                                    isOutput=False)
    y_out = nc.declare_dram_parameter("y", [GROUPS, 128, FREE], F16,
                                      isOutput=True)

    xb = [nc.alloc_sbuf_tensor(f"xb{i}", [128, F], F16).ap() for i in range(2)]
    tp = [nc.alloc_sbuf_tensor(f"tp{i}", [128, F], F32R).ap() for i in range(RT)]
    pp = [nc.alloc_sbuf_tensor(f"pp{i}", [128, F], F32R).ap() for i in range(RP)]
    rb = [nc.alloc_sbuf_tensor(f"rb{i}", [128, F], F16).ap() for i in range(RR)]
    yb = [nc.alloc_sbuf_tensor(f"yb{i}", [128, F], F16).ap() for i in range(RY)]
    pb = nc.alloc_sbuf_tensor("pb", [128, 64], F32).ap()
    wgb = nc.alloc_sbuf_tensor("wgb", [128, NW * 128], F32).ap()
    wgr = nc.alloc_sbuf_tensor("wgr", [128, NW * 128], F32R).ap()
    wgh = nc.alloc_sbuf_tensor("wgh", [128, T_A * 128], F16).ap()
    ps = [nc.alloc_psum_tensor(f"ps{i}", [128, F], F32).ap() for i in range(2)]

    a_s = pb[:, 0:1]

    def unit_slice(u):
        g, ci = divmod(u, FREE // F)
        return g, ci * F

    def w_ap(widx):
        return wgr[:, 128 * widx:128 * (widx + 1)]

    with (nc.Block() as block,
          nc.semaphore("s_pr") as s_pr,      # prologue wgt DMAs
          nc.semaphore("s_pp") as s_pp,      # prm DMA
          nc.semaphore("s_in") as s_in,      # x tile DMAs
          nc.semaphore("s_i0") as s_i0,      # unit-0 first-half input
          nc.semaphore("s_wg") as s_wg,      # wgr f32r copy done
          nc.semaphore("s_tp") as s_tp,      # t-streams done
          nc.semaphore("s_rt") as s_rt,      # relu streams done
          nc.semaphore("s_dve") as s_dve,    # partials done
          nc.semaphore("s_mm") as s_mm,      # PE merges done (1/merge)
          nc.semaphore("s_cp") as s_cp,      # PSUM->yb copies done
          nc.semaphore("s_out") as s_out):   # output DMAs done

        @block.sync
        def _(sync):
            def dma_in(u):
                g, off = unit_slice(u)
                if u >= 2:
                    # xb[u%2] free once unit u-2's ACT reads retired
                    sync.wait_ge(s_tp, u - 1)
                    sync.wait_ge(s_rt, T_A * (u - 1))
                sync.dma_start(out=xb[u % 2][:],
                               in_=x_in[g, :, off:off + F]).then_inc(s_in, 16)

            # unit-0 load split with the gpsimd queue (second half there)
            g0, off0 = unit_slice(0)
            sync.dma_start(out=xb[0][:, 0:F // 2],
                           in_=x_in[g0, :, off0:off0 + F // 2]
                           ).then_inc(s_i0, 16)
            sync.dma_start(out=pb[:], in_=prm[:, :]).then_inc(s_pp, 16)
            dma_in(1)
            for u in range(UNITS - 1):
                if u == 0:
                    for k in range(NW):
                        sync.dma_start(out=wgb[:, 128 * k:128 * (k + 1)],
                                       in_=wgt[k, :, :]).then_inc(s_pr, 16)
                if u + 2 < UNITS:
                    dma_in(u + 2)
                sync.wait_ge(s_cp, u + 1)
                g, off = unit_slice(u)
                sync.dma_start(out=y_out[g, :, off:off + F],
                               in_=yb[u % RY][:]).then_inc(s_out, 16)
            uL = UNITS - 1
            gL, offL = unit_slice(uL)
            for h_ in range(2):
                sync.wait_ge(s_cp, uL + 1 + h_)
                sync.dma_start(
                    out=y_out[gL, :, offL + 1024 * h_:offL + 1024 * (h_ + 1)],
                    in_=yb[uL % RY][:, 1024 * h_:1024 * (h_ + 1)]
                    ).then_inc(s_out, 16)

        @block.gpsimd
        def _(gp):
            g0, off0 = unit_slice(0)
            gp.dma_start(out=xb[0][:, F // 2:F],
                         in_=x_in[g0, :, off0 + F // 2:off0 + F]
                         ).then_inc(s_in, 16)

        @block.scalar
        def _(scalar):
            def act_t(u):
                # t = a_s*x + 25
                if u == 0:
                    scalar.wait_ge(s_pp, 16)
                    scalar.wait_ge(s_i0, 16)
                scalar.wait_ge(s_in, 16 * (u + 1))
                if u >= RT:
                    # tp[u%RT] free: partials + seed merge of u-RT done
                    scalar.wait_ge(s_dve, K3 * (u - RT + 1))
                    scalar.wait_ge(s_mm, M * (u - RT) + 1)
                scalar.activation(out=tp[u % RT][:], in_=xb[u % 2][:],
                                  func=Ident, scale=a_s,
                                  bias=pb[:, 1:2]).then_inc(s_tp, 1)

            def act_r(u):
                for j in range(T_A):
                    if u == 0 and j == 6:
                        # one-time: round the weights for the PE (deferred
                        # so the serialized wgt DMAs don't gate t'(0);
                        # placed before the rb ring wraps at j=8)
                        scalar.wait_ge(s_pr, 16 * NW)
                        scalar.activation(out=wgr[:], in_=wgb[:], func=Ident,
                                          scale=1.0, bias=pb[:, 63:64])
                        scalar.activation(out=wgh[:], in_=wgb[:, 2 * 128:],
                                          func=Ident, scale=1.0,
                                          bias=pb[:, 63:64]).then_inc(s_wg, 1)
                    gidx = u * T_A + j
                    if gidx >= RR:
                        u2, j2 = divmod(gidx - RR, T_A)
                        scalar.wait_ge(s_mm, M * u2 + MI_R[j2] + 1)
                    scalar.activation(out=rb[gidx % RR][:], in_=xb[u % 2][:],
                                      func=Relu, scale=a_s,
                                      bias=pb[:, 42 + j:43 + j]
                                      ).then_inc(s_rt, 1)

            def copy_out(u):
                # yb = psum + A  (fp16 out)
                scalar.wait_ge(s_mm, M * (u + 1))
                if u >= RY:
                    scalar.wait_ge(s_out, 16 * (u - RY + 1))
                scalar.activation(out=yb[u % RY][:], in_=ps[u % 2][:],
                                  func=Ident, scale=1.0,
                                  bias=pb[:, 2:3]).then_inc(s_cp, 1)

            for u in range(UNITS):
                act_t(u)
                act_r(u)
                if u >= 1 and not (ALT_COPY and (u - 1) % 2 == 0):
                    copy_out(u - 1)
            # unit 15 drain: copy in halves gated on the split final merge
            uL = UNITS - 1
            for h_ in range(2):
                scalar.wait_ge(s_mm, M * uL + (M - 1) + h_ + 1)
                scalar.activation(out=yb[uL % RY][:, 1024 * h_:1024 * (h_ + 1)],
                                  in_=ps[uL % 2][:, 1024 * h_:1024 * (h_ + 1)],
                                  func=Ident, scale=1.0,
                                  bias=pb[:, 2:3]).then_inc(s_cp, 1)

        @block.vector
        def _(vector):
            vector.wait_ge(s_pp, 16)

            def dve_copy(u):
                vector.wait_ge(s_mm, M * (u + 1))
                if u >= RY:
                    vector.wait_ge(s_out, 16 * (u - RY + 1))
                vector.tensor_scalar(out=yb[u % RY][:], in0=ps[u % 2][:],
                                     scalar1=pb[:, 2:3], scalar2=None,
                                     op0=mybir.AluOpType.add
                                     ).then_inc(s_cp, 1)

            for u in range(UNITS):
                vector.wait_ge(s_tp, u + 1)
                for i in range(K3):
                    gidx = u * K3 + i
                    if gidx >= RP:
                        u2, i2 = divmod(gidx - RP, K3)
                        vector.wait_ge(s_mm, M * u2 + MI_P[i2] + 1)
                    if u == UNITS - 1 and i == K3 - 1:
                        # drain: last partial in halves so the final merge
                        # chain starts earlier
                        for lo, hi in ((0, F // 2), (F // 2, F)):
                            vector._custom_dve(
                                P3L, out=pp[gidx % RP][:, lo:hi],
                                in0=tp[u % RT].bitcast(
                                    mybir.dt.float32)[:, lo:hi],
                                in1=pb[:, 5 + 3 * i:6 + 3 * i],
                                s0=pb[:, 3 + 3 * i:4 + 3 * i],
                                s1=pb[:, 4 + 3 * i:5 + 3 * i],
                                imm2=float(3 * i + 2)).then_inc(s_dve, 1)
                        continue
                    vector._custom_dve(
                        P3L, out=pp[gidx % RP][:],
                        in0=tp[u % RT].bitcast(mybir.dt.float32)[:],
                        in1=pb[:, 5 + 3 * i:6 + 3 * i],      # C3 spill
                        s0=pb[:, 3 + 3 * i:4 + 3 * i],
                        s1=pb[:, 4 + 3 * i:5 + 3 * i],
                        imm2=float(3 * i + 2)).then_inc(s_dve, 1)
                if ALT_COPY and u >= 1 and (u - 1) % 2 == 0:
                    dve_copy(u - 1)
            if ALT_COPY and (UNITS - 1) % 2 == 0:
                dve_copy(UNITS - 1)

        @block.tensor
        def _(tensor):
            tensor.wait_ge(s_wg, 1)
            for u in range(UNITS):
                for m, (kind, j) in enumerate(MO):
                    if kind == "s":
                        tensor.wait_ge(s_tp, u + 1)
                        if u >= 2:
                            tensor.wait_ge(s_cp, u - 1)   # psum tile free
                        widx, src = 0, tp[u % RT]
                    elif kind == "r":
                        tensor.wait_ge(s_rt, u * T_A + j + 1)
                        widx, src = None, rb[(u * T_A + j) % RR]
                        w_r = wgh[:, 128 * j:128 * (j + 1)]
                    else:
                        if not (u == UNITS - 1 and j == K3 - 1):
                            tensor.wait_ge(s_dve, u * K3 + j + 1)
                        widx, src = 1, pp[(u * K3 + j) % RP]
                    lhs = w_ap(widx) if widx is not None else w_r
                    last_drain = (u == UNITS - 1 and m == M - 1)
                    for b_ in range(NB):
                        if last_drain and b_ == 0:
                            tensor.wait_ge(s_dve, (UNITS - 1) * K3 + K3)
                        elif last_drain and b_ == 2:
                            tensor.wait_ge(s_dve, (UNITS - 1) * K3 + K3 + 1)
                        sl = slice(512 * b_, 512 * (b_ + 1))
                        mm = tensor.matmul(ps[u % 2][:, sl], lhs,
                                           src[:, sl],
                                           start=(m == 0), stop=(m == M - 1),
                                           skip_group_check=True)
                        if last_drain and b_ == 1:
                            mm.then_inc(s_mm, 1)
                        elif b_ == NB - 1:
                            mm.then_inc(s_mm, 1)

    lower_extended_insts(nc)
    _built["nc"] = nc
    return nc


def kernel(x, coefficients_vect, scaling_coeffs_vect):
    from concourse.bass_utils import run_bass_kernel_spmd
    from concourse import bass2jax
    bass2jax.install_neuronx_cc_hook()

    x = np.asarray(x, _f32)
    coeff = np.asarray(coefficients_vect, _f32).reshape(-1)
    scal = np.asarray(scaling_coeffs_vect, _f32).reshape(-1)

    prm_full, wg = _host_params(coeff, scal)

    nb = N_BATCH // NCORES                          # 4 batches per core
    x16 = x.astype(np.float16)
    in_maps = []
    for i in range(NCORES):
        xi = x16[nb * i:nb * (i + 1)].reshape(GROUPS, 128, FREE)
        in_maps.append({"x": np.ascontiguousarray(xi), "prm": prm_full,
                        "wgt": wg})

    nc = _build()
    res = run_bass_kernel_spmd(nc, in_maps, list(range(NCORES)))

    out = np.empty((N_BATCH, N_CH, H, W), _f32)
    for i in range(NCORES):
        out[nb * i:nb * (i + 1)] = np.asarray(
            res.results[i]["y"]).astype(_f32).reshape(nb, N_CH, H, W)
    return out
